# revision 8
# baseline (speedup 1.0000x reference)
"""Trainium2 Bass kernel for nn_Compressor (NSA-style windowed KV compression).

Math (per reference):
  kv   = x @ wkv_w.T                     [B, S, 1024]
  gate = sigmoid(x @ wgate_w.T)
  kv   = kv * gate + tile(ape)           (ape per position-within-window)
  kv   = mean over windows of 4          [B, S/4, 2, 512]
  out  = norm_w * kv * rsqrt(mean(kv^2, -1) + eps)   [B, S/2, 512]

Distribution: x flattened to [B*S, 4096] = [16384, 4096], sharded into 8
contiguous 2048-row blocks (whole windows per shard); weights replicated.
Each core computes its [1024, 512] output shard; host concatenates.

On-chip strategy (per core):
  - kv matmul in fp16 (accurate); gate matmul in fp8(e4m3) using the PE's
    DoubleRow perf mode (0.5 cycles/row, 2x fp16 throughput, K=256 per
    instruction). The sigmoid damps the fp8 quantization error:
    d(kv*g) = kv*g*(1-g)*dz, so end-to-end rel err ~1.5e-2 < 2e-2.
    fp8 operands are pre-scaled on host (x*8, wg.T*1024) to sit in e4m3's
    normal range; the 1/8192 is folded into the sigmoid's activation scale.
  - PE work per 128-row j-tile (both 512-col chunks together): 32 full-M
    (M=128) DoubleRow gate matmuls (N=512, K=256 per column -> half the
    streamed columns of fp16) then 64 fp16 kv matmuls, c0/c1 interleaved
    so consecutive matmuls alternate psum banks and share each
    stationary x chunk.
  - both weight matrices resident in SBUF; x streamed once (fp16 + fp8).
  - psum: 2x kv [128,512] single-buffered + 2x gate [128,512]
    double-buffered (tile i+1's gate matmuls need not wait for tile i's
    sigmoid reads) + double-buffered pool-matmul bank pair = 8 banks.
  - epilogue: 2 sigmoids (ACT, scale=1/8192) + DVE cross-quadrant copy
    + gate-mul + ape-add (DVE), window-pool via a PE matmul against a
    [128, 32] 0.25-indicator matrix, RMSNorm on the free dim, DMA out.
    Epilogues are deferred by one j-tile so pool matmuls and epilogue
    engine work hide under the next tile's matmuls.
"""

import sys

sys.path.insert(0, "/opt/trn_rl_repo")

import numpy as np
import ml_dtypes

import concourse.tile as tile
from concourse import bacc, mybir
from concourse.bass_utils import run_bass_kernel_spmd

HALF = np.float16
FP8 = ml_dtypes.float8_e4m3

N_CORES = 8
B, S, D = 4, 4096, 4096
R = 4                  # compress ratio (window)
HD = 512               # head dim
OD = 1024              # coff * head_dim
EPS = 1e-6

ROWS = (B * S) // N_CORES      # 2048 sequence rows per core
DC = D // 128                  # 32 contraction chunks
NT = ROWS // 128               # 16 s-tiles per core
SBLK = 256                     # x columns loaded per DMA block (2 s-tiles)
NW_TILE = 128 // R             # 32 windows per s-tile

SX = 8.0                       # fp8 pre-scale for x
SW = 1024.0                    # fp8 pre-scale for wgate
GSCL = 1.0 / (SX * SW)         # folded into sigmoid activation scale

_CACHED_NC = None


def _build_nc(reps=1):
    nc = bacc.Bacc("TRN2", target_bir_lowering=False, debug=False,
                   num_devices=N_CORES)
    f32 = mybir.dt.float32
    f16 = mybir.dt.float16
    f8 = mybir.dt.float8e4

    xt = nc.dram_tensor("xt", [D, ROWS], f16, kind="ExternalInput").ap()
    x8 = nc.dram_tensor("x8", [D, ROWS], f8, kind="ExternalInput").ap()
    wkvt = nc.dram_tensor("wkvt", [D, OD], f16, kind="ExternalInput").ap()
    wg8 = nc.dram_tensor("wg8", [D, OD], f8, kind="ExternalInput").ap()
    apeb = nc.dram_tensor("apeb", [128, OD], f32, kind="ExternalInput").ap()
    nrmb = nc.dram_tensor("nrmb", [128, HD], f32, kind="ExternalInput").ap()
    poolm = nc.dram_tensor("poolm", [128, NW_TILE], f16, kind="ExternalInput").ap()
    out = nc.dram_tensor("out", [ROWS // R * 2, HD], f32, kind="ExternalOutput").ap()

    # [p, dc, n] views with the contraction dim on partitions
    xt_v = xt.rearrange("(dc p) s -> p dc s", p=128)
    x8_v = x8.rearrange("(dc p) s -> p dc s", p=128)
    wkvt_v = wkvt.rearrange("(dc p) o -> p dc o", p=128)
    wg8_v = wg8.rearrange("(dc p) o -> p dc o", p=128)
    out_v = out.rearrange("(w two) h -> w two h", two=2)

    with tile.TileContext(nc) as tc:
        with (
            tc.tile_pool(name="const", bufs=1) as const_pool,
            tc.tile_pool(name="wpool", bufs=1) as wpool,
            tc.tile_pool(name="xpool", bufs=2) as xpool,
            tc.tile_pool(name="acts", bufs=2) as acts,
            tc.tile_pool(name="small", bufs=2) as small,
            tc.tile_pool(name="mm", bufs=1, space="PSUM") as psum_pool,
            tc.tile_pool(name="gmm", bufs=2, space="PSUM") as gpsum_pool,
            tc.tile_pool(name="pl", bufs=2, space="PSUM") as pool_psum,
        ):
            WSL = 2   # dc chunks per weight DMA slice (= one DoubleRow kg)

            wkv_sl, wg_sl = [], []
            for s0 in range(DC // WSL):
                t = wpool.tile([128, WSL, OD], f16, tag=f"wkv{s0}")
                nc.sync.dma_start(t[:], wkvt_v[:, s0 * WSL:(s0 + 1) * WSL, :])
                wkv_sl.append(t)
                t = wpool.tile([128, WSL, OD], f8, tag=f"wg{s0}")
                nc.sync.dma_start(t[:], wg8_v[:, s0 * WSL:(s0 + 1) * WSL, :])
                wg_sl.append(t)

            apeb_sb = const_pool.tile([128, OD], f32)
            nc.sync.dma_start(apeb_sb[:], apeb)
            nrmb_sb = const_pool.tile([128, HD], f32)
            nc.sync.dma_start(nrmb_sb[:], nrmb)
            poolm_sb = const_pool.tile([128, NW_TILE], f16)
            nc.sync.dma_start(poolm_sb[:], poolm)
            eps_sb = const_pool.tile([128, 1], f32)
            nc.gpsimd.memset(eps_sb[:], EPS)

            def load_xblk(blk):
                t16 = xpool.tile([128, DC, SBLK], f16, tag="xtb")
                nc.sync.dma_start(
                    t16[:], xt_v[:, :, blk * SBLK:(blk + 1) * SBLK])
                t8 = xpool.tile([128, DC, SBLK], f8, tag="x8b")
                nc.sync.dma_start(
                    t8[:], x8_v[:, :, blk * SBLK:(blk + 1) * SBLK])
                return t16, t8

            def epilogue(ps_kv, ps_g, i, c):
                # ps_g is [128, 512] (full-M DoubleRow output): one sigmoid.
                gate_sb = acts.tile([128, HD], f32, tag="gate")
                nc.scalar.activation(gate_sb[:], ps_g[:],
                                     mybir.ActivationFunctionType.Sigmoid,
                                     scale=GSCL)
                kvg_sb = acts.tile([128, HD], f32, tag="kvg")
                nc.vector.tensor_mul(kvg_sb[:], ps_kv[:], gate_sb[:])
                nc.vector.tensor_add(kvg_sb[:], kvg_sb[:],
                                     apeb_sb[:, c * HD:(c + 1) * HD])
                kvg16 = acts.tile([128, HD], f16, tag="kvg16")
                nc.vector.tensor_copy(kvg16[:], kvg_sb[:])
                pooled_ps = pool_psum.tile([NW_TILE, HD], f32, tag="pooled")
                nc.tensor.matmul(pooled_ps[:], poolm_sb[:], kvg16[:],
                                 start=True, stop=True)
                # RMSNorm over the free (head) dim
                pooled_sb = small.tile([NW_TILE, HD], f32, tag="pooled_sb")
                nc.vector.tensor_copy(pooled_sb[:], pooled_ps[:])
                sqj = small.tile([NW_TILE, HD], f32, tag="sqj")
                ssq = small.tile([NW_TILE, 1], f32, tag="ssq")
                nc.vector.tensor_mul(sqj[:], pooled_sb[:], pooled_sb[:])
                nc.vector.reduce_sum(ssq[:], sqj[:], axis=mybir.AxisListType.X)
                std = small.tile([NW_TILE, 1], f32, tag="std")
                nc.scalar.activation(std[:], ssq[:],
                                     mybir.ActivationFunctionType.Sqrt,
                                     bias=eps_sb[:NW_TILE, :], scale=1.0 / HD)
                rinv = small.tile([NW_TILE, 1], f32, tag="rinv")
                nc.vector.reciprocal(rinv[:], std[:])
                onorm = small.tile([NW_TILE, HD], f32, tag="onorm")
                nc.scalar.mul(onorm[:], pooled_sb[:], rinv[:])
                nc.vector.tensor_mul(onorm[:], onorm[:], nrmb_sb[:NW_TILE, :])
                nc.sync.dma_start(
                    out_v[i * NW_TILE:(i + 1) * NW_TILE, c, :], onorm[:])

            pending = []

            def flush(keep):
                while len(pending) > keep:
                    epilogue(*pending.pop(0))

            for _rep in range(reps):
                for blk in range(NT * 128 // SBLK):
                    xtb, x8b = load_xblk(blk)
                    for j in range(SBLK // 128):
                        i = blk * (SBLK // 128) + j
                        # Both c-chunks of the j-tile processed together:
                        # gates first (their psums are ready early so the
                        # ACT sigmoids overlap the kv matmuls), then the kv
                        # matmuls for c0/c1 interleaved -- consecutive
                        # matmuls alternate psum banks (same-bank
                        # accumulation is ~11% slower) and share each
                        # stationary x chunk.
                        # Gate: full-M (128) DoubleRow, N=512 per c-chunk:
                        # each streamed column carries K=256, so the gate
                        # costs half the columns of an fp16 equivalent.
                        ps_gs = []
                        for c in range(2):
                            ps_g = gpsum_pool.tile([128, HD], f32,
                                                   tag=f"ps_g{c}")
                            ps_gs.append(ps_g)
                        for kg in range(DC // 2):
                            lhsT = x8b[:, 2 * kg:2 * kg + 2,
                                       j * 128:(j + 1) * 128]
                            for c in range(2):
                                nc.tensor.matmul(
                                    ps_gs[c][:], lhsT,
                                    wg_sl[kg][:, :, c * HD:(c + 1) * HD],
                                    start=(kg == 0),
                                    stop=(kg == DC // 2 - 1),
                                    perf_mode=
                                    mybir.MatmulPerfMode.DoubleRow)
                        ps_k0 = psum_pool.tile([128, HD], f32, tag="ps_k0")
                        ps_k1 = psum_pool.tile([128, HD], f32, tag="ps_k1")
                        for dc in range(DC):
                            lhsT = xtb[:, dc, j * 128:(j + 1) * 128]
                            for c, ps_kv in ((0, ps_k0), (1, ps_k1)):
                                nc.tensor.matmul(
                                    ps_kv[:], lhsT,
                                    wkv_sl[dc // WSL][:, dc % WSL,
                                                      c * HD:(c + 1) * HD],
                                    start=(dc == 0), stop=(dc == DC - 1))
                        pending.append((ps_k0, ps_gs[0], i, 0))
                        pending.append((ps_k1, ps_gs[1], i, 1))
                        flush(2)
            flush(0)

    nc.compile()
    return nc


def _get_nc():
    global _CACHED_NC
    if _CACHED_NC is None:
        _CACHED_NC = _build_nc()
    return _CACHED_NC


def _prep_in_maps(x, wkv_w, wgate_w, ape, norm_w):
    x = np.asarray(x, dtype=np.float32)
    wkv_w = np.asarray(wkv_w, dtype=np.float32)
    wgate_w = np.asarray(wgate_w, dtype=np.float32)
    ape = np.asarray(ape, dtype=np.float32)
    norm_w = np.asarray(norm_w, dtype=np.float32)

    xb = x.reshape(B * S, D)
    xb16 = xb.astype(HALF)
    xb8 = (xb * SX).astype(FP8)
    wkvt = np.ascontiguousarray(wkv_w.astype(HALF).T)             # [D, OD]
    wg8 = np.ascontiguousarray((wgate_w * SW).astype(FP8).T)      # [D, OD]
    apeb = np.ascontiguousarray(np.tile(ape, (128 // R, 1)))      # [128, OD]
    nrmb = np.ascontiguousarray(np.tile(norm_w[None, :], (128, 1)))  # [128, HD]
    poolm = np.zeros((128, NW_TILE), np.float32)
    poolm[np.arange(128), np.arange(128) // R] = 1.0 / R
    poolm = poolm.astype(HALF)

    in_maps = []
    for k in range(N_CORES):
        xt_k = np.ascontiguousarray(xb16[k * ROWS:(k + 1) * ROWS, :].T)
        x8_k = np.ascontiguousarray(xb8[k * ROWS:(k + 1) * ROWS, :].T)
        in_maps.append({
            "xt": xt_k, "x8": x8_k, "wkvt": wkvt, "wg8": wg8,
            "apeb": apeb, "nrmb": nrmb, "poolm": poolm,
        })
    return in_maps


def kernel(x, wkv_w, wgate_w, ape, norm_w):
    nc = _get_nc()
    in_maps = _prep_in_maps(x, wkv_w, wgate_w, ape, norm_w)
    try:
        res = run_bass_kernel_spmd(nc, in_maps, list(range(N_CORES)))
    except Exception:
        # Transient axon-transport failures are retryable; a wedged device
        # (NRT_EXEC_UNIT_UNRECOVERABLE) recovers with a fresh PJRT session.
        try:
            import jax
            jax.clear_backends()
        except Exception:
            pass
        res = run_bass_kernel_spmd(nc, in_maps, list(range(N_CORES)))
    shards = [res.results[k]["out"] for k in range(N_CORES)]
    return np.concatenate(shards, axis=0).reshape(B, S // R * 2, HD)



# revision 16
# speedup vs baseline: 10.8255x; 10.8255x over previous
"""Trainium2 Bass kernel for nn_Compressor (NSA-style windowed KV compression).

Math (per reference):
  kv   = x @ wkv_w.T                     [B, S, 1024]
  gate = sigmoid(x @ wgate_w.T)
  kv   = kv * gate + tile(ape)           (ape per position-within-window)
  kv   = mean over windows of 4          [B, S/4, 2, 512]
  out  = norm_w * kv * rsqrt(mean(kv^2, -1) + eps)   [B, S/2, 512]

Distribution: x flattened to [B*S, 4096] = [16384, 4096], sharded into 8
contiguous 2048-row blocks (whole windows per shard); weights replicated.
Each core computes its [1024, 512] output shard; host concatenates.

On-chip strategy (per core):
  - kv matmul in fp16 (accurate); gate matmul in fp8(e4m3) using the PE's
    DoubleRow perf mode (0.5 cycles/row, 2x fp16 throughput, K=256 per
    instruction). The sigmoid damps the fp8 quantization error:
    d(kv*g) = kv*g*(1-g)*dz, so end-to-end rel err ~1.5e-2 < 2e-2.
    fp8 operands are pre-scaled on host (x*8, wg.T*1024) to sit in e4m3's
    normal range; the 1/8192 is folded into the sigmoid's activation scale.
  - PE work per 128-row j-tile (both 512-col chunks together): 32 full-M
    (M=128) DoubleRow gate matmuls (N=512, K=256 per column -> half the
    streamed columns of fp16) then 64 fp16 kv matmuls, c0/c1 interleaved
    so consecutive matmuls alternate psum banks and share each
    stationary x chunk.
  - both weight matrices resident in SBUF; x streamed once (fp16 + fp8).
  - psum: 2x kv [128,512] single-buffered + 2x gate [128,512]
    double-buffered (tile i+1's gate matmuls need not wait for tile i's
    sigmoid reads) + double-buffered pool-matmul bank pair = 8 banks.
  - epilogue: 2 sigmoids (ACT, scale=1/8192) + DVE cross-quadrant copy
    + gate-mul + ape-add (DVE), window-pool via a PE matmul against a
    [128, 32] 0.25-indicator matrix, RMSNorm on the free dim, DMA out.
    Epilogues are deferred by one j-tile so pool matmuls and epilogue
    engine work hide under the next tile's matmuls.
"""

import sys

sys.path.insert(0, "/opt/trn_rl_repo")

import numpy as np
import ml_dtypes

import concourse.tile as tile
from concourse import bacc, mybir
from concourse.bass_utils import run_bass_kernel_spmd

HALF = np.float16
FP8 = ml_dtypes.float8_e4m3

N_CORES = 8
B, S, D = 4, 4096, 4096
R = 4                  # compress ratio (window)
HD = 512               # head dim
OD = 1024              # coff * head_dim
EPS = 1e-6

ROWS = (B * S) // N_CORES      # 2048 sequence rows per core
DC = D // 128                  # 32 contraction chunks
NT = ROWS // 128               # 16 s-tiles per core
SBLK = 256                     # x columns loaded per DMA block (2 s-tiles)
NW_TILE = 128 // R             # 32 windows per s-tile

SX = 8.0                       # fp8 pre-scale for x
SW = 1024.0                    # fp8 pre-scale for wgate
GSCL = 1.0 / (SX * SW)         # folded into sigmoid activation scale
G8 = 2                         # kv contraction chunk-PAIRS done in fp8 DR
# The kv psum accumulates fp8 (x*8 @ wkv*1024) and fp16 partials in one
# bank, so the fp16 operands are pre-scaled by the same 8/1024 factors;
# the whole kv path then carries an 8192x scale that RMSNorm absorbs
# (eps scaled by 8192^2; the final rsqrt divides the 8192 back out).
KVS = SX * SW                  # 8192

_CACHED_NC = None


def _build_nc(reps=1):
    nc = bacc.Bacc("TRN2", target_bir_lowering=False, debug=False,
                   num_devices=N_CORES)
    f32 = mybir.dt.float32
    f16 = mybir.dt.float16
    f8 = mybir.dt.float8e4

    xt = nc.dram_tensor("xt", [D, ROWS], f16, kind="ExternalInput").ap()
    x8 = nc.dram_tensor("x8", [D, ROWS], f8, kind="ExternalInput").ap()
    wkvt = nc.dram_tensor("wkvt", [D, OD], f16, kind="ExternalInput").ap()
    wk8 = nc.dram_tensor("wk8", [G8 * 256, OD], f8, kind="ExternalInput").ap()
    wg8 = nc.dram_tensor("wg8", [D, OD], f8, kind="ExternalInput").ap()
    apeb = nc.dram_tensor("apeb", [128, OD], f32, kind="ExternalInput").ap()
    nrmb = nc.dram_tensor("nrmb", [128, HD], f32, kind="ExternalInput").ap()
    poolm = nc.dram_tensor("poolm", [128, NW_TILE], f16, kind="ExternalInput").ap()
    out = nc.dram_tensor("out", [ROWS // R * 2, HD], f32, kind="ExternalOutput").ap()

    # [p, dc, n] views with the contraction dim on partitions
    xt_v = xt.rearrange("(dc p) s -> p dc s", p=128)
    x8_v = x8.rearrange("(dc p) s -> p dc s", p=128)
    wkvt_v = wkvt.rearrange("(dc p) o -> p dc o", p=128)
    wk8_v = wk8.rearrange("(dc p) o -> p dc o", p=128)
    wg8_v = wg8.rearrange("(dc p) o -> p dc o", p=128)
    out_v = out.rearrange("(w two) h -> w two h", two=2)

    with tile.TileContext(nc) as tc:
        with (
            tc.tile_pool(name="const", bufs=1) as const_pool,
            tc.tile_pool(name="wpool", bufs=1) as wpool,
            tc.tile_pool(name="xpool", bufs=2) as xpool,
            tc.tile_pool(name="acts", bufs=2) as acts,
            tc.tile_pool(name="small", bufs=2) as small,
            tc.tile_pool(name="mm", bufs=1, space="PSUM") as psum_pool,
            tc.tile_pool(name="gmm", bufs=2, space="PSUM") as gpsum_pool,
            tc.tile_pool(name="pl", bufs=2, space="PSUM") as pool_psum,
        ):
            WSL = 2   # dc chunks per weight DMA slice (= one DoubleRow kg)

            wkv_sl, wg_sl, wk8_sl = [], [], []
            for s0 in range(DC // WSL):
                t = wpool.tile([128, WSL, OD], f16, tag=f"wkv{s0}")
                nc.sync.dma_start(t[:], wkvt_v[:, s0 * WSL:(s0 + 1) * WSL, :])
                wkv_sl.append(t)
                t = wpool.tile([128, WSL, OD], f8, tag=f"wg{s0}")
                nc.sync.dma_start(t[:], wg8_v[:, s0 * WSL:(s0 + 1) * WSL, :])
                wg_sl.append(t)
            for s0 in range(G8):
                t = wpool.tile([128, 2, OD], f8, tag=f"wk8{s0}")
                nc.sync.dma_start(t[:], wk8_v[:, s0 * 2:(s0 + 1) * 2, :])
                wk8_sl.append(t)

            apeb_sb = const_pool.tile([128, OD], f32)
            nc.sync.dma_start(apeb_sb[:], apeb)
            nrmb_sb = const_pool.tile([128, HD], f32)
            nc.sync.dma_start(nrmb_sb[:], nrmb)
            poolm_sb = const_pool.tile([128, NW_TILE], f16)
            nc.sync.dma_start(poolm_sb[:], poolm)
            eps_sb = const_pool.tile([128, 1], f32)
            nc.gpsimd.memset(eps_sb[:], EPS * KVS * KVS)

            def load_xblk(blk):
                t16 = xpool.tile([128, DC, SBLK], f16, tag="xtb")
                nc.sync.dma_start(
                    t16[:], xt_v[:, :, blk * SBLK:(blk + 1) * SBLK])
                t8 = xpool.tile([128, DC, SBLK], f8, tag="x8b")
                nc.sync.dma_start(
                    t8[:], x8_v[:, :, blk * SBLK:(blk + 1) * SBLK])
                return t16, t8

            def epilogue(ps_kv, ps_g, i, c):
                # ps_g is [128, 512] (full-M DoubleRow output): one sigmoid.
                gate_sb = acts.tile([128, HD], f32, tag="gate")
                nc.scalar.activation(gate_sb[:], ps_g[:],
                                     mybir.ActivationFunctionType.Sigmoid,
                                     scale=GSCL)
                kvg_sb = acts.tile([128, HD], f32, tag="kvg")
                nc.vector.tensor_mul(kvg_sb[:], ps_kv[:], gate_sb[:])
                nc.vector.tensor_add(kvg_sb[:], kvg_sb[:],
                                     apeb_sb[:, c * HD:(c + 1) * HD])
                kvg16 = acts.tile([128, HD], f16, tag="kvg16")
                nc.vector.tensor_copy(kvg16[:], kvg_sb[:])
                pooled_ps = pool_psum.tile([NW_TILE, HD], f32, tag="pooled")
                nc.tensor.matmul(pooled_ps[:], poolm_sb[:], kvg16[:],
                                 start=True, stop=True)
                # RMSNorm over the free (head) dim
                pooled_sb = small.tile([NW_TILE, HD], f32, tag="pooled_sb")
                nc.vector.tensor_copy(pooled_sb[:], pooled_ps[:])
                sqj = small.tile([NW_TILE, HD], f32, tag="sqj")
                ssq = small.tile([NW_TILE, 1], f32, tag="ssq")
                nc.vector.tensor_mul(sqj[:], pooled_sb[:], pooled_sb[:])
                nc.vector.reduce_sum(ssq[:], sqj[:], axis=mybir.AxisListType.X)
                std = small.tile([NW_TILE, 1], f32, tag="std")
                nc.scalar.activation(std[:], ssq[:],
                                     mybir.ActivationFunctionType.Sqrt,
                                     bias=eps_sb[:NW_TILE, :], scale=1.0 / HD)
                rinv = small.tile([NW_TILE, 1], f32, tag="rinv")
                nc.vector.reciprocal(rinv[:], std[:])
                onorm = small.tile([NW_TILE, HD], f32, tag="onorm")
                nc.scalar.mul(onorm[:], pooled_sb[:], rinv[:])
                nc.vector.tensor_mul(onorm[:], onorm[:], nrmb_sb[:NW_TILE, :])
                nc.sync.dma_start(
                    out_v[i * NW_TILE:(i + 1) * NW_TILE, c, :], onorm[:])

            pending = []

            def flush(keep):
                while len(pending) > keep:
                    epilogue(*pending.pop(0))

            for _rep in range(reps):
                for blk in range(NT * 128 // SBLK):
                    xtb, x8b = load_xblk(blk)
                    for j in range(SBLK // 128):
                        i = blk * (SBLK // 128) + j
                        # Both c-chunks of the j-tile processed together:
                        # gates first (their psums are ready early so the
                        # ACT sigmoids overlap the kv matmuls), then the kv
                        # matmuls for c0/c1 interleaved -- consecutive
                        # matmuls alternate psum banks (same-bank
                        # accumulation is ~11% slower) and share each
                        # stationary x chunk.
                        # Gate: full-M (128) DoubleRow, N=512 per c-chunk:
                        # each streamed column carries K=256, so the gate
                        # costs half the columns of an fp16 equivalent.
                        ps_gs = []
                        for c in range(2):
                            ps_g = gpsum_pool.tile([128, HD], f32,
                                                   tag=f"ps_g{c}")
                            ps_gs.append(ps_g)
                        for kg in range(DC // 2):
                            lhsT = x8b[:, 2 * kg:2 * kg + 2,
                                       j * 128:(j + 1) * 128]
                            for c in range(2):
                                nc.tensor.matmul(
                                    ps_gs[c][:], lhsT,
                                    wg_sl[kg][:, :, c * HD:(c + 1) * HD],
                                    start=(kg == 0),
                                    stop=(kg == DC // 2 - 1),
                                    perf_mode=
                                    mybir.MatmulPerfMode.DoubleRow)
                        # kv: first G8 chunk-pairs in fp8 DoubleRow (err
                        # budget allows ~1/8 of the contraction in fp8;
                        # saves one 216ns column-stream per pair per c),
                        # remainder in fp16.
                        ps_k0 = psum_pool.tile([128, HD], f32, tag="ps_k0")
                        ps_k1 = psum_pool.tile([128, HD], f32, tag="ps_k1")
                        for kg in range(G8):
                            lhsT = x8b[:, 2 * kg:2 * kg + 2,
                                       j * 128:(j + 1) * 128]
                            for c, ps_kv in ((0, ps_k0), (1, ps_k1)):
                                nc.tensor.matmul(
                                    ps_kv[:], lhsT,
                                    wk8_sl[kg][:, :, c * HD:(c + 1) * HD],
                                    start=(kg == 0), stop=False,
                                    perf_mode=mybir.MatmulPerfMode.DoubleRow)
                        for dc in range(2 * G8, DC):
                            lhsT = xtb[:, dc, j * 128:(j + 1) * 128]
                            for c, ps_kv in ((0, ps_k0), (1, ps_k1)):
                                nc.tensor.matmul(
                                    ps_kv[:], lhsT,
                                    wkv_sl[dc // WSL][:, dc % WSL,
                                                      c * HD:(c + 1) * HD],
                                    start=False, stop=(dc == DC - 1))
                        pending.append((ps_k0, ps_gs[0], i, 0))
                        pending.append((ps_k1, ps_gs[1], i, 1))
                        flush(2)
            flush(0)

    nc.compile()
    return nc


def _get_nc():
    global _CACHED_NC
    if _CACHED_NC is None:
        _CACHED_NC = _build_nc()
    return _CACHED_NC


def _prep_in_maps(x, wkv_w, wgate_w, ape, norm_w):
    x = np.asarray(x, dtype=np.float32)
    wkv_w = np.asarray(wkv_w, dtype=np.float32)
    wgate_w = np.asarray(wgate_w, dtype=np.float32)
    ape = np.asarray(ape, dtype=np.float32)
    norm_w = np.asarray(norm_w, dtype=np.float32)

    xb = x.reshape(B * S, D)
    xb16 = (xb * SX).astype(HALF)
    xb8 = (xb * SX).astype(FP8)
    wkvt = np.ascontiguousarray((wkv_w * SW).astype(HALF).T)      # [D, OD]
    wk8 = np.ascontiguousarray(
        (wkv_w[:, :G8 * 256] * SW).astype(FP8).T)                 # [G8*256, OD]
    wg8 = np.ascontiguousarray((wgate_w * SW).astype(FP8).T)      # [D, OD]
    apeb = np.ascontiguousarray(
        np.tile(ape * (SX * SW), (128 // R, 1)))                  # [128, OD]
    nrmb = np.ascontiguousarray(np.tile(norm_w[None, :], (128, 1)))  # [128, HD]
    poolm = np.zeros((128, NW_TILE), np.float32)
    poolm[np.arange(128), np.arange(128) // R] = 1.0 / R
    poolm = poolm.astype(HALF)

    in_maps = []
    for k in range(N_CORES):
        xt_k = np.ascontiguousarray(xb16[k * ROWS:(k + 1) * ROWS, :].T)
        x8_k = np.ascontiguousarray(xb8[k * ROWS:(k + 1) * ROWS, :].T)
        in_maps.append({
            "xt": xt_k, "x8": x8_k, "wkvt": wkvt, "wk8": wk8, "wg8": wg8,
            "apeb": apeb, "nrmb": nrmb, "poolm": poolm,
        })
    return in_maps


def kernel(x, wkv_w, wgate_w, ape, norm_w):
    nc = _get_nc()
    in_maps = _prep_in_maps(x, wkv_w, wgate_w, ape, norm_w)
    try:
        res = run_bass_kernel_spmd(nc, in_maps, list(range(N_CORES)))
    except Exception:
        # Transient axon-transport failures are retryable; a wedged device
        # (NRT_EXEC_UNIT_UNRECOVERABLE) recovers with a fresh PJRT session.
        try:
            import jax
            jax.clear_backends()
        except Exception:
            pass
        res = run_bass_kernel_spmd(nc, in_maps, list(range(N_CORES)))
    shards = [res.results[k]["out"] for k in range(N_CORES)]
    return np.concatenate(shards, axis=0).reshape(B, S // R * 2, HD)

